# revision 1
# baseline (speedup 1.0000x reference)
"""Trainium2 Bass kernel for nn_DecoderSmoothedMaxPoolingLoss.

Loss (see reference):
  neg  = -log(1 - X)                                    (B,T,K)
  loss = sum_{b, t<len_b, k} neg
         - sum_{b, i in [0,Lw_b), k=tgt_b} neg[b, tau_s_b + i, k]
         + sum_b -log( max_j  clip(conv_same(win_b * valid_b, filt), EPS, 1) * valid_b )
  where tau_s = max(0, w_end + 40 - 60), tau_e = min(tau_s + 60, len),
  Lw = tau_e - tau_s, win_b[i] = X[b, tau_s_b + i, tgt_b].

Sharding: pure data parallel over batch — 8 batches per core on 8 cores.
Each core computes its partial scalar loss on device; host sums the 8
partials (the "all-reduce").

Per core (8 local batches = 12.8 MB):
  big term:  16 half-batch chunks, each viewed flat as (125, 1600)
             [t = 2000*half + 16p + j, k inner].  SWDGE DMA (measured
             ~167 GB/s at this chunk size vs ~116 for HWDGE), ACT Ln
             in place, then ONE fused DVE tensor_tensor_reduce per
             chunk: multiply by the host mask (-1 where t < len, else
             0; broadcast across k via a step-0 AP) and accumulate to
             a per-partition scalar -> C[:, chunk].
  windows:   one indirect DMA gathers 60 contiguous rows per batch
             (8 descriptors x 24KB) into Wp (8, 6000); one-hot select
             of k = tgt via host-built ohrep -> win_raw (8,60).
             Exclusion = sum valid * Ln(1-win_raw).  Positive term:
             conv as two small matmuls (transpose via identity, then
             win_v @ M), clip, mask, row-max, Ln.  Emitted after the
             chunk loads so the gather doesn't head-block the SWDGE
             FIFO.
  final:     partition sums via matmuls with a ones vector, free
             reduce -> scalar out.
"""

import numpy as np

import concourse.bass as bass
import concourse.tile as tile
from concourse import bacc
from concourse import mybir
from concourse import bass_utils
from concourse.bass import IndirectOffsetOnAxis

AF = mybir.ActivationFunctionType
ALU = mybir.AluOpType
AX = mybir.AxisListType
FP = mybir.dt.float32
I32 = mybir.dt.int32

B, T, K = 64, 4000, 100
WIN, OFFSET_D, TRUNC, SIGMA = 60, 40, 21, 9
EPS = 1e-8
NCORES = 8
BLOC = B // NCORES          # 8 batches per core
NCH = 16                    # half-batch chunks per core
PROWS = 125                 # chunk viewed (125, 1600); t = 2000*h + 16p + j
JCOLS = 16
FREE = JCOLS * K            # 1600

# packed aux layouts
WA = WIN * K + WIN + BLOC           # auxA (8, 6068): ohrep | valid8 | I8
WB = WIN + NCH * JCOLS              # auxB (125, 316): M (rows<60) | maskH


def _filt_np():
    half = TRUNC // 2
    x = np.arange(-half, half + 1, dtype=np.float32)
    g = np.exp(-0.5 * (x / SIGMA) ** 2).astype(np.float32)
    g = g / g.sum()
    f = np.zeros(WIN, np.float32)
    c = WIN // 2
    f[c - half:c + half + 1] = g
    return f


def _conv_matrix():
    # smoothed[j] = sum_i win[i] * filt[i - j + pl], pl = (WIN-1)//2
    f = _filt_np()
    pl = (WIN - 1) // 2
    idx = np.arange(WIN)
    u = idx[:, None] - idx[None, :] + pl          # (i, j)
    M = np.where((u >= 0) & (u < WIN), f[np.clip(u, 0, WIN - 1)], 0.0)
    return M.astype(np.float32)


_NC_CACHE = None


def _build_program():
    global _NC_CACHE
    if _NC_CACHE is not None:
        return _NC_CACHE

    nc = bacc.Bacc("TRN2", debug=False)
    Xs = nc.dram_tensor("Xs", [BLOC, T, K], FP, kind="ExternalInput").ap()
    auxA = nc.dram_tensor("auxA", [BLOC, WA], FP, kind="ExternalInput").ap()
    auxB = nc.dram_tensor("auxB", [PROWS, WB], FP, kind="ExternalInput").ap()
    gofs = nc.dram_tensor("gofs", [BLOC, 1], I32, kind="ExternalInput").ap()
    outd = nc.dram_tensor("out", [1, 1], FP, kind="ExternalOutput").ap()

    with tile.TileContext(nc) as tc:
        with tc.tile_pool(name="xin", bufs=NCH) as xin_pool, \
             tc.tile_pool(name="small", bufs=1) as small, \
             tc.tile_pool(name="psum", bufs=1, space="PSUM") as psum:

            # ---------- aux loads (HWDGE; SWDGE is reserved for bulk) ----
            gofs_sb = small.tile([BLOC, 1], I32)
            nc.sync.dma_start(out=gofs_sb[:], in_=gofs)
            auxA_sb = small.tile([BLOC, WA], FP)
            nc.sync.dma_start(out=auxA_sb[:], in_=auxA)
            auxB_sb = small.tile([PROWS, WB], FP)
            nc.sync.dma_start(out=auxB_sb[:], in_=auxB)

            ohrep_sl = auxA_sb[0:BLOC, 0:WIN * K]
            valid_sl = auxA_sb[0:BLOC, WIN * K:WIN * K + WIN]
            I8_sl = auxA_sb[0:BLOC, WIN * K + WIN:WA]
            M_sl = auxB_sb[0:WIN, 0:WIN]
            maskH_sl = auxB_sb[0:PROWS, WIN:WB]

            C = small.tile([PROWS, NCH], FP)

            # ---------- big term: 16 half-batch chunks ----------
            Xflat = Xs.rearrange("b t k -> (b t k)")
            CH = PROWS * FREE
            xtiles = []
            for c in range(NCH):
                xb = xin_pool.tile([PROWS, FREE], FP, tag="xb")
                nc.gpsimd.dma_start(
                    out=xb[:],
                    in_=Xflat[c * CH:(c + 1) * CH].rearrange(
                        "(p f) -> p f", p=PROWS))
                xtiles.append(xb)
            for c in range(NCH):
                xb = xtiles[c]
                nc.scalar.activation(out=xb[:], in_=xb[:], func=AF.Ln,
                                     bias=1.0, scale=-1.0)
                xv = xb[:].rearrange("p (j k) -> p j k", k=K)
                mv = maskH_sl[:, c * JCOLS:(c + 1) * JCOLS]
                mb = bass.AP(tensor=mv.tensor, offset=mv.offset,
                             ap=[mv.ap[0], mv.ap[1], [0, K]])
                nc.vector.tensor_tensor(out=xv, in0=xv, in1=mb, op=ALU.mult)
                nc.vector.tensor_reduce(out=C[0:PROWS, c:c + 1], in_=xb[:],
                                        axis=AX.X, op=ALU.add)

            # ---------- window path ----------
            Wp = small.tile([BLOC, WIN * K], FP)
            nc.gpsimd.indirect_dma_start(
                out=Wp[:],
                out_offset=None,
                in_=Xs.rearrange("b t k -> (b t) k"),
                in_offset=IndirectOffsetOnAxis(ap=gofs_sb[:, :1], axis=0),
            )
            nc.vector.tensor_tensor(out=Wp[:], in0=Wp[:], in1=ohrep_sl,
                                    op=ALU.mult)
            win_raw = small.tile([BLOC, WIN], FP)
            nc.vector.tensor_reduce(
                out=win_raw[:],
                in_=Wp[:].rearrange("b (i k) -> b i k", k=K),
                axis=AX.X, op=ALU.add)
            # positive term part 1: win_v = win_raw * valid
            winv = small.tile([BLOC, WIN], FP)
            nc.vector.tensor_tensor(out=winv[:], in0=win_raw[:],
                                    in1=valid_sl, op=ALU.mult)
            # exclusion: + sum_i valid * ln(1 - win_raw)
            lnw = small.tile([BLOC, WIN], FP)
            nc.scalar.activation(out=lnw[:], in_=win_raw[:], func=AF.Ln,
                                 bias=1.0, scale=-1.0)
            lnwv = small.tile([BLOC, WIN], FP)
            nc.vector.tensor_tensor(out=lnwv[:], in0=lnw[:], in1=valid_sl,
                                    op=ALU.mult)
            exclcol = small.tile([BLOC, 1], FP)
            nc.vector.tensor_reduce(out=exclcol[:], in_=lnwv[:],
                                    axis=AX.X, op=ALU.add)
            # smoothed = win_v @ M (transpose first via identity)
            wvt_ps = psum.tile([WIN, BLOC], FP)
            nc.tensor.matmul(out=wvt_ps[:], lhsT=winv[:], rhs=I8_sl,
                             start=True, stop=True)
            wvt = small.tile([WIN, BLOC], FP)
            nc.vector.tensor_copy(out=wvt[:], in_=wvt_ps[:])
            sm_ps = psum.tile([BLOC, WIN], FP)
            nc.tensor.matmul(out=sm_ps[:], lhsT=wvt[:], rhs=M_sl,
                             start=True, stop=True)
            smc = small.tile([BLOC, WIN], FP)
            nc.vector.tensor_scalar(out=smc[:], in0=sm_ps[:],
                                    scalar1=EPS, scalar2=1.0,
                                    op0=ALU.max, op1=ALU.min)
            smv = small.tile([BLOC, WIN], FP)
            nc.vector.tensor_tensor(out=smv[:], in0=smc[:], in1=valid_sl,
                                    op=ALU.mult)
            mx = small.tile([BLOC, 1], FP)
            nc.vector.tensor_reduce(out=mx[:], in_=smv[:], axis=AX.X,
                                    op=ALU.max)
            lnmx = small.tile([BLOC, 1], FP)
            nc.scalar.activation(out=lnmx[:], in_=mx[:], func=AF.Ln)
            poscol = small.tile([BLOC, 1], FP)
            nc.vector.tensor_scalar_mul(poscol[:], lnmx[:], -1.0)

            # ---------- final partition reduce ----------
            ones = small.tile([PROWS, 1], FP)
            nc.vector.memset(ones[:], 1.0)
            tot_ps = psum.tile([1, NCH + 2], FP)
            nc.tensor.matmul(out=tot_ps[:, 0:NCH], lhsT=ones[:], rhs=C[:],
                             start=True, stop=True)
            nc.tensor.matmul(out=tot_ps[:, NCH:NCH + 1],
                             lhsT=ones[0:BLOC, :], rhs=exclcol[:],
                             start=True, stop=True)
            nc.tensor.matmul(out=tot_ps[:, NCH + 1:NCH + 2],
                             lhsT=ones[0:BLOC, :], rhs=poscol[:],
                             start=True, stop=True)
            tot = small.tile([1, 1], FP)
            nc.vector.tensor_reduce(out=tot[:], in_=tot_ps[:], axis=AX.X,
                                    op=ALU.add)
            nc.gpsimd.dma_start(out=outd, in_=tot[:])

    nc.compile()
    _NC_CACHE = nc
    return nc


def _make_in_maps(X, lengths, tgt, w_end):
    X = np.ascontiguousarray(np.asarray(X, dtype=np.float32))
    lengths = np.asarray(lengths, dtype=np.int64)
    tgt = np.asarray(tgt, dtype=np.int64)
    w_end = np.asarray(w_end, dtype=np.int64)

    tau_s = np.maximum(0, w_end + OFFSET_D - WIN)
    tau_e = np.minimum(tau_s + WIN, lengths)
    Lw = tau_e - tau_s

    # t for chunk c at (p, j): 2000*(c%2) + 16p + j
    tloc = 16 * np.arange(PROWS)[:, None] + np.arange(JCOLS)[None, :]
    Mmat = _conv_matrix()
    I8 = np.eye(BLOC, dtype=np.float32)

    in_maps = []
    for cr in range(NCORES):
        bs = slice(cr * BLOC, (cr + 1) * BLOC)
        ls, ts, lw, tg = lengths[bs], tau_s[bs], Lw[bs], tgt[bs]

        oh = np.zeros((BLOC, K), np.float32)
        oh[np.arange(BLOC), tg] = 1.0
        ohrep = np.broadcast_to(oh[:, None, :], (BLOC, WIN, K)) \
            .reshape(BLOC, WIN * K)
        valid8 = (np.arange(WIN)[None, :] < lw[:, None]).astype(np.float32)
        auxA = np.concatenate([ohrep, valid8, I8], axis=1)  # (8, WA)

        # maskH: (125, 16 chunks * 16 cols), -1 where t < len
        cols = []
        for c in range(NCH):
            b, h = c // 2, c % 2
            t = 2000 * h + tloc
            cols.append(np.where(t < ls[b], np.float32(-1.0),
                                 np.float32(0.0)))
        maskH = np.concatenate(cols, axis=1)                # (125, 256)
        Mpad = np.zeros((PROWS, WIN), np.float32)
        Mpad[0:WIN] = Mmat
        auxB = np.concatenate([Mpad, maskH], axis=1)        # (125, WB)

        gofs_arr = (np.arange(BLOC) * T + ts).astype(np.int32) \
            .reshape(BLOC, 1)
        in_maps.append({
            "Xs": np.ascontiguousarray(X[bs]),
            "auxA": np.ascontiguousarray(auxA),
            "auxB": np.ascontiguousarray(auxB),
            "gofs": gofs_arr,
        })
    return in_maps


def kernel(X, lengths, tgt, w_end):
    nc = _build_program()
    in_maps = _make_in_maps(X, lengths, tgt, w_end)
    res = bass_utils.run_bass_kernel_spmd(
        nc, in_maps, core_ids=list(range(NCORES)))
    total = np.float32(0.0)
    for c in range(NCORES):
        total += np.float32(res.results[c]["out"][0, 0])
    return np.array(total, dtype=np.float32)



# revision 10
# speedup vs baseline: 1.7849x; 1.7849x over previous
"""Trainium2 Bass kernel for nn_DecoderSmoothedMaxPoolingLoss.

Loss (see reference):
  neg  = -log(1 - X)                                    (B,T,K)
  loss = sum_{b, t<len_b, k} neg
         - sum_{b, i in [0,Lw_b), k=tgt_b} neg[b, tau_s_b + i, k]
         + sum_b -log( max_j  clip(conv_same(win_b * valid_b, filt), EPS, 1) * valid_b )
  where tau_s = max(0, w_end + 40 - 60), tau_e = min(tau_s + 60, len),
  Lw = tau_e - tau_s, win_b[i] = X[b, tau_s_b + i, tgt_b].

Sharding: pure data parallel over batch — 8 batches per core on 8 cores.
Each core computes its partial scalar loss on device; host sums the 8
partials (the "all-reduce").

Per core (8 local batches = 12.8 MB, viewed flat as (128, 25000)):
  host prep:  the per-core X copy is written with its invalid tail
              (t >= len_b) zeroed, so ln(1-0) = 0 contributes nothing
              and no length mask is needed on device at all.
  big term:   6 descending-size chunks (128, F) on the sync HWDGE ring
              (contiguous F*4-byte descriptors per partition, >= 2 KB,
              mostly 12-25 KB -> near-peak HBM rate).  Per chunk: ACT
              Ln(1-x) in place, DVE tensor_reduce -> column of C.
  windows:    one indirect DMA per core gathers 60 contiguous frames per
              batch -> Wp (8, 6000); one-hot select of k = tgt via
              host-built ohrep (DVE mult + reduce; DVE is mostly idle).
              Conv as two small matmuls; clip/mask/max tiny DVE ops.
  final:      all partial columns live in C (128, NCOL), with a host
              +-1 weight row fixing signs; one matmul with a ones
              vector -> (1, NCOL) PSUM, weight-multiply + reduce ->
              scalar, DMA out.
"""

import numpy as np

import concourse.bass as bass
import concourse.tile as tile
from concourse import bacc
from concourse import mybir
from concourse import bass_utils
from concourse.bass import IndirectOffsetOnAxis

AF = mybir.ActivationFunctionType
ALU = mybir.AluOpType
AX = mybir.AxisListType
FP = mybir.dt.float32
I32 = mybir.dt.int32

B, T, K = 64, 4000, 100
WIN, OFFSET_D, TRUNC, SIGMA = 60, 40, 21, 9
EPS = 1e-8
NCORES = 8
BLOC = B // NCORES          # 8 batches per core
P = 128                     # SBUF partitions
FTOT = BLOC * T * K // P    # 25000 floats per partition
FCH = [6250, 6250, 5000, 4000, 3000, 500]   # descending chunk sizes
NCH = len(FCH)
assert sum(FCH) == FTOT
GLEN = WIN * K              # 6000 gathered floats per batch window
NCOL = NCH + 2              # C columns: chunk sums | excl | pos
AUXW = 2 * WIN + BLOC + NCOL  # aux cols: M | valid8 | I8 | wrow


def _filt_np():
    half = TRUNC // 2
    x = np.arange(-half, half + 1, dtype=np.float32)
    g = np.exp(-0.5 * (x / SIGMA) ** 2).astype(np.float32)
    g = g / g.sum()
    f = np.zeros(WIN, np.float32)
    c = WIN // 2
    f[c - half:c + half + 1] = g
    return f


def _conv_matrix():
    # smoothed[j] = sum_i win[i] * filt[i - j + pl], pl = (WIN-1)//2
    f = _filt_np()
    pl = (WIN - 1) // 2
    idx = np.arange(WIN)
    u = idx[:, None] - idx[None, :] + pl          # (i, j)
    M = np.where((u >= 0) & (u < WIN), f[np.clip(u, 0, WIN - 1)], 0.0)
    return M.astype(np.float32)


_NC_CACHE = None


def _build_program():
    global _NC_CACHE
    if _NC_CACHE is not None:
        return _NC_CACHE

    nc = bacc.Bacc("TRN2", debug=False)
    Xs = nc.dram_tensor("Xs", [P, FTOT], FP, kind="ExternalInput").ap()
    aux = nc.dram_tensor("aux", [WIN, AUXW], FP, kind="ExternalInput").ap()
    auxO = nc.dram_tensor("auxO", [BLOC, GLEN], FP,
                          kind="ExternalInput").ap()
    gofs = nc.dram_tensor("gofs", [BLOC, 1], I32, kind="ExternalInput").ap()
    outd = nc.dram_tensor("out", [1, 1], FP, kind="ExternalOutput").ap()

    with tile.TileContext(nc) as tc:
        with tc.tile_pool(name="xin", bufs=1) as xin_pool, \
             tc.tile_pool(name="small", bufs=1) as small, \
             tc.tile_pool(name="psum", bufs=1, space="PSUM") as psum:

            # ---- small loads first on the sync ring (cheap dispatches) ----
            gofs_sb = small.tile([BLOC, 1], I32)
            nc.sync.dma_start(out=gofs_sb[:], in_=gofs)
            aux_sb = small.tile([WIN, AUXW], FP)
            nc.sync.dma_start(out=aux_sb[:], in_=aux)
            ohrep_sb = small.tile([BLOC, GLEN], FP)
            nc.sync.dma_start(out=ohrep_sb[:], in_=auxO)

            M_sl = aux_sb[0:WIN, 0:WIN]
            valid_sl = aux_sb[0:BLOC, WIN:2 * WIN]
            I8_sl = aux_sb[0:BLOC, 2 * WIN:2 * WIN + BLOC]
            wrow_sl = aux_sb[0:1, 2 * WIN + BLOC:AUXW]

            # ---- bulk chunk loads on the sync HWDGE ring ----
            xtiles = []
            base = 0
            for ci, F in enumerate(FCH):
                xb = xin_pool.tile([P, F], FP, tag=f"xb{ci}",
                                   name=f"xb{ci}")
                nc.sync.dma_start(out=xb[:], in_=Xs[:, base:base + F])
                xtiles.append(xb)
                base += F

            # ---- window gather (SWDGE indirect, row-resolved) ----
            Wp = small.tile([BLOC, GLEN], FP)
            nc.gpsimd.indirect_dma_start(
                out=Wp[:],
                out_offset=None,
                in_=Xs.rearrange("p (r k) -> (p r) k", k=K),
                in_offset=IndirectOffsetOnAxis(ap=gofs_sb[:, :1], axis=0),
            )

            C = small.tile([P, NCOL], FP)
            nc.vector.memset(C[:], 0.0)
            ones = small.tile([P, 1], FP)
            nc.vector.memset(ones[:], 1.0)

            # ---- big term: Ln(1-x) in place + per-partition sum ----
            for ci in range(NCH):
                xb = xtiles[ci]
                nc.scalar.activation(out=xb[:], in_=xb[:], func=AF.Ln,
                                     bias=1.0, scale=-1.0)
                nc.vector.tensor_reduce(out=C[0:P, ci:ci + 1], in_=xb[:],
                                        axis=AX.X, op=ALU.add)
                if ci == 0:
                    # one-hot select: win_raw[b,i] = sum_k Wp*ohrep
                    nc.vector.tensor_tensor(out=Wp[:], in0=Wp[:],
                                            in1=ohrep_sb[:], op=ALU.mult)
                    win_raw = small.tile([BLOC, WIN], FP)
                    nc.vector.tensor_reduce(
                        out=win_raw[:],
                        in_=Wp[:].rearrange("b (i k) -> b i k", k=K),
                        axis=AX.X, op=ALU.add)
                    # exclusion: ln(1-win_raw), * valid, row-sum
                    lnw = small.tile([BLOC, WIN], FP)
                    nc.scalar.activation(out=lnw[:], in_=win_raw[:],
                                         func=AF.Ln, bias=1.0, scale=-1.0)
                    lnwv = small.tile([BLOC, WIN], FP)
                    nc.vector.tensor_tensor(out=lnwv[:], in0=lnw[:],
                                            in1=valid_sl, op=ALU.mult)
                    nc.vector.tensor_reduce(out=C[0:BLOC, NCH:NCH + 1],
                                            in_=lnwv[:], axis=AX.X,
                                            op=ALU.add)
                    # winv = win_raw * valid
                    winv = small.tile([BLOC, WIN], FP)
                    nc.vector.tensor_tensor(out=winv[:], in0=win_raw[:],
                                            in1=valid_sl, op=ALU.mult)
                    # conv: transpose winv via matmul with I8, then @ M
                    wvt_ps = psum.tile([WIN, BLOC], FP)
                    nc.tensor.matmul(out=wvt_ps[:], lhsT=winv[:],
                                     rhs=I8_sl, start=True, stop=True)
                    wvt = small.tile([WIN, BLOC], FP)
                    nc.vector.tensor_copy(out=wvt[:], in_=wvt_ps[:])
                    sm_ps = psum.tile([BLOC, WIN], FP)
                    nc.tensor.matmul(out=sm_ps[:], lhsT=wvt[:], rhs=M_sl,
                                     start=True, stop=True)
                    # clip to [EPS, 1]
                    smc = small.tile([BLOC, WIN], FP)
                    nc.vector.tensor_scalar(out=smc[:], in0=sm_ps[:],
                                            scalar1=EPS, scalar2=1.0,
                                            op0=ALU.max, op1=ALU.min)
                    # mask + row max
                    smv = small.tile([BLOC, WIN], FP)
                    nc.vector.tensor_tensor(out=smv[:], in0=smc[:],
                                            in1=valid_sl, op=ALU.mult)
                    mx = small.tile([BLOC, 1], FP)
                    nc.vector.tensor_reduce(out=mx[:], in_=smv[:],
                                            axis=AX.X, op=ALU.max)

            # pos col: ln(mx) per batch
            nc.scalar.activation(out=C[0:BLOC, NCH + 1:NCH + 2], in_=mx[:],
                                 func=AF.Ln)

            # ---- final: tot = sum over columns of wrow * colsum ----
            tot_ps = psum.tile([1, NCOL], FP)
            nc.tensor.matmul(out=tot_ps[:], lhsT=ones[:], rhs=C[:],
                             start=True, stop=True)
            negrow = small.tile([1, NCOL], FP)
            nc.vector.tensor_tensor(out=negrow[:], in0=tot_ps[:],
                                    in1=wrow_sl, op=ALU.mult)
            tot = small.tile([1, 1], FP)
            nc.vector.tensor_reduce(out=tot[:], in_=negrow[:], axis=AX.X,
                                    op=ALU.add)
            nc.gpsimd.dma_start(out=outd, in_=tot[:])

    nc.compile()
    _NC_CACHE = nc
    return nc


def _make_in_maps(X, lengths, tgt, w_end):
    X = np.asarray(X, dtype=np.float32)
    lengths = np.asarray(lengths, dtype=np.int64)
    tgt = np.asarray(tgt, dtype=np.int64)
    w_end = np.asarray(w_end, dtype=np.int64)

    tau_s = np.maximum(0, w_end + OFFSET_D - WIN)
    tau_e = np.minimum(tau_s + WIN, lengths)
    Lw = tau_e - tau_s

    Mmat = _conv_matrix()

    # final-combine weights: big cols and pos get -1, excl gets +1
    # (C holds +sum ln everywhere; loss = -A + Ex - L)
    wrow = np.full(NCOL, -1.0, np.float32)
    wrow[NCH] = 1.0

    in_maps = []
    for cr in range(NCORES):
        bs = slice(cr * BLOC, (cr + 1) * BLOC)
        ls, ts, lw, tg = lengths[bs], tau_s[bs], Lw[bs], tgt[bs]

        # per-core X copy with the invalid tail zeroed
        Xc = np.array(X[bs])                     # (8, T, K) contiguous copy
        for b in range(BLOC):
            lb = int(ls[b])
            if lb < T:
                Xc[b, lb:] = 0.0

        valid8 = (np.arange(WIN)[None, :] < lw[:, None]).astype(np.float32)
        aux = np.zeros((WIN, AUXW), np.float32)
        aux[0:WIN, 0:WIN] = Mmat
        aux[0:BLOC, WIN:2 * WIN] = valid8
        aux[0:BLOC, 2 * WIN:2 * WIN + BLOC] = np.eye(BLOC, dtype=np.float32)
        aux[0, 2 * WIN + BLOC:AUXW] = wrow

        oh = np.zeros((BLOC, K), np.float32)
        oh[np.arange(BLOC), tg] = 1.0
        ohrep = np.broadcast_to(oh[:, None, :], (BLOC, WIN, K)) \
            .reshape(BLOC, GLEN)

        gofs_arr = (np.arange(BLOC) * T + ts).astype(np.int32) \
            .reshape(BLOC, 1)
        in_maps.append({
            "Xs": Xc.reshape(P, FTOT),
            "aux": aux,
            "auxO": np.ascontiguousarray(ohrep),
            "gofs": gofs_arr,
        })
    return in_maps


def kernel(X, lengths, tgt, w_end):
    nc = _build_program()
    in_maps = _make_in_maps(X, lengths, tgt, w_end)
    res = bass_utils.run_bass_kernel_spmd(
        nc, in_maps, core_ids=list(range(NCORES)))
    total = np.float32(0.0)
    for c in range(NCORES):
        total += np.float32(res.results[c]["out"][0, 0])
    return np.array(total, dtype=np.float32)


# revision 12
# speedup vs baseline: 2.0983x; 1.1756x over previous
"""Trainium2 Bass kernel for nn_DecoderSmoothedMaxPoolingLoss.

Loss (see reference):
  neg  = -log(1 - X)                                    (B,T,K)
  loss = sum_{b, t<len_b, k} neg
         - sum_{b, i in [0,Lw_b), k=tgt_b} neg[b, tau_s_b + i, k]
         + sum_b -log( max_j  clip(conv_same(win_b * valid_b, filt), EPS, 1) * valid_b )
  where tau_s = max(0, w_end + 40 - 60), tau_e = min(tau_s + 60, len),
  Lw = tau_e - tau_s, win_b[i] = X[b, tau_s_b + i, tgt_b].

Sharding: pure data parallel over batch — 8 batches per core on 8 cores.
Each core computes its partial scalar loss on device; host sums the 8
partials (the "all-reduce").

Per core (8 local batches = 12.8 MB, viewed flat as (128, 25000)):
  host prep:  the per-core X copy is written with its invalid tail
              (t >= len_b) zeroed, so ln(1-0) = 0 contributes nothing
              and no length mask is needed on device at all.
  big term:   6 descending-size chunks (128, F) on the sync HWDGE ring
              (contiguous F*4-byte descriptors per partition, >= 2 KB,
              mostly 12-25 KB -> near-peak HBM rate).  Per chunk: ACT
              Ln(1-x) in place, DVE tensor_reduce -> column of C.
  windows:    one indirect DMA per core gathers 60 contiguous frames per
              batch -> Wp (8, 6000); one-hot select of k = tgt via
              host-built ohrep (DVE mult + reduce; DVE is mostly idle).
              Conv as two small matmuls; clip/mask/max tiny DVE ops.
  final:      all partial columns live in C (128, NCOL), with a host
              +-1 weight row fixing signs; one matmul with a ones
              vector -> (1, NCOL) PSUM, weight-multiply + reduce ->
              scalar, DMA out.
"""

import numpy as np

import concourse.bass as bass
import concourse.tile as tile
from concourse import bacc
from concourse import mybir
from concourse import bass_utils
from concourse.bass import IndirectOffsetOnAxis

AF = mybir.ActivationFunctionType
ALU = mybir.AluOpType
AX = mybir.AxisListType
FP = mybir.dt.float32
I32 = mybir.dt.int32

B, T, K = 64, 4000, 100
WIN, OFFSET_D, TRUNC, SIGMA = 60, 40, 21, 9
EPS = 1e-8
NCORES = 8
BLOC = B // NCORES          # 8 batches per core
P = 128                     # SBUF partitions
FTOT = BLOC * T * K // P    # 25000 floats per partition
FCH = [6250, 6250, 5000, 4000, 3000, 500]   # descending chunk sizes
NCH = len(FCH)
assert sum(FCH) == FTOT
GLEN = WIN * K              # 6000 gathered floats per batch window
NCOL = NCH + 2              # C columns: chunk sums | excl | pos
AUXW = 2 * WIN + BLOC + NCOL  # aux cols: M | valid8 | I8 | wrow


def _filt_np():
    half = TRUNC // 2
    x = np.arange(-half, half + 1, dtype=np.float32)
    g = np.exp(-0.5 * (x / SIGMA) ** 2).astype(np.float32)
    g = g / g.sum()
    f = np.zeros(WIN, np.float32)
    c = WIN // 2
    f[c - half:c + half + 1] = g
    return f


def _conv_matrix():
    # smoothed[j] = sum_i win[i] * filt[i - j + pl], pl = (WIN-1)//2
    f = _filt_np()
    pl = (WIN - 1) // 2
    idx = np.arange(WIN)
    u = idx[:, None] - idx[None, :] + pl          # (i, j)
    M = np.where((u >= 0) & (u < WIN), f[np.clip(u, 0, WIN - 1)], 0.0)
    return M.astype(np.float32)


_NC_CACHE = None


def _build_program():
    global _NC_CACHE
    if _NC_CACHE is not None:
        return _NC_CACHE

    nc = bacc.Bacc("TRN2", debug=False)
    Xs = nc.dram_tensor("Xs", [P, FTOT], FP, kind="ExternalInput").ap()
    aux = nc.dram_tensor("aux", [WIN, AUXW], FP, kind="ExternalInput").ap()
    auxO = nc.dram_tensor("auxO", [BLOC, GLEN], FP,
                          kind="ExternalInput").ap()
    gofs = nc.dram_tensor("gofs", [BLOC, 1], I32, kind="ExternalInput").ap()
    outd = nc.dram_tensor("out", [1, 1], FP, kind="ExternalOutput").ap()

    with tile.TileContext(nc) as tc:
        with tc.tile_pool(name="xin", bufs=1) as xin_pool, \
             tc.tile_pool(name="small", bufs=1) as small, \
             tc.tile_pool(name="psum", bufs=1, space="PSUM") as psum:

            # ---- bulk chunk loads on the sync HWDGE ring, dispatched
            # first so streaming starts as early as possible ----
            xtiles = []
            base = 0
            for ci, F in enumerate(FCH):
                xb = xin_pool.tile([P, F], FP, tag=f"xb{ci}",
                                   name=f"xb{ci}")
                nc.sync.dma_start(out=xb[:], in_=Xs[:, base:base + F])
                xtiles.append(xb)
                base += F

            # ---- small loads on the gpsimd (SWDGE) ring in parallel ----
            gofs_sb = small.tile([BLOC, 1], I32)
            nc.gpsimd.dma_start(out=gofs_sb[:], in_=gofs)
            aux_sb = small.tile([WIN, AUXW], FP)
            nc.gpsimd.dma_start(out=aux_sb[:], in_=aux)
            ohrep_sb = small.tile([BLOC, GLEN], FP)
            nc.gpsimd.dma_start(out=ohrep_sb[:], in_=auxO)

            M_sl = aux_sb[0:WIN, 0:WIN]
            valid_sl = aux_sb[0:BLOC, WIN:2 * WIN]
            I8_sl = aux_sb[0:BLOC, 2 * WIN:2 * WIN + BLOC]
            wrow_sl = aux_sb[0:1, 2 * WIN + BLOC:AUXW]

            # ---- window gather (SWDGE indirect, row-resolved) ----
            Wp = small.tile([BLOC, GLEN], FP)
            nc.gpsimd.indirect_dma_start(
                out=Wp[:],
                out_offset=None,
                in_=Xs.rearrange("p (r k) -> (p r) k", k=K),
                in_offset=IndirectOffsetOnAxis(ap=gofs_sb[:, :1], axis=0),
            )

            C = small.tile([P, NCOL], FP)
            nc.vector.memset(C[:], 0.0)
            ones = small.tile([P, 1], FP)
            nc.vector.memset(ones[:], 1.0)

            # ---- big term: Ln(1-x) in place + fused per-partition sum ----
            for ci in range(NCH):
                xb = xtiles[ci]
                nc.scalar.activation(out=xb[:], in_=xb[:], func=AF.Ln,
                                     bias=1.0, scale=-1.0,
                                     accum_out=C[0:P, ci:ci + 1])
                if ci == 0:
                    # one-hot select: win_raw[b,i] = sum_k Wp*ohrep
                    nc.vector.tensor_tensor(out=Wp[:], in0=Wp[:],
                                            in1=ohrep_sb[:], op=ALU.mult)
                    win_raw = small.tile([BLOC, WIN], FP)
                    nc.vector.tensor_reduce(
                        out=win_raw[:],
                        in_=Wp[:].rearrange("b (i k) -> b i k", k=K),
                        axis=AX.X, op=ALU.add)
                    # exclusion: ln(1-win_raw), * valid, row-sum
                    lnw = small.tile([BLOC, WIN], FP)
                    nc.scalar.activation(out=lnw[:], in_=win_raw[:],
                                         func=AF.Ln, bias=1.0, scale=-1.0)
                    lnwv = small.tile([BLOC, WIN], FP)
                    nc.vector.tensor_tensor(out=lnwv[:], in0=lnw[:],
                                            in1=valid_sl, op=ALU.mult)
                    nc.vector.tensor_reduce(out=C[0:BLOC, NCH:NCH + 1],
                                            in_=lnwv[:], axis=AX.X,
                                            op=ALU.add)
                    # winv = win_raw * valid
                    winv = small.tile([BLOC, WIN], FP)
                    nc.vector.tensor_tensor(out=winv[:], in0=win_raw[:],
                                            in1=valid_sl, op=ALU.mult)
                    # conv: transpose winv via matmul with I8, then @ M
                    wvt_ps = psum.tile([WIN, BLOC], FP)
                    nc.tensor.matmul(out=wvt_ps[:], lhsT=winv[:],
                                     rhs=I8_sl, start=True, stop=True)
                    wvt = small.tile([WIN, BLOC], FP)
                    nc.vector.tensor_copy(out=wvt[:], in_=wvt_ps[:])
                    sm_ps = psum.tile([BLOC, WIN], FP)
                    nc.tensor.matmul(out=sm_ps[:], lhsT=wvt[:], rhs=M_sl,
                                     start=True, stop=True)
                    # clip to [EPS, 1]
                    smc = small.tile([BLOC, WIN], FP)
                    nc.vector.tensor_scalar(out=smc[:], in0=sm_ps[:],
                                            scalar1=EPS, scalar2=1.0,
                                            op0=ALU.max, op1=ALU.min)
                    # mask + row max
                    smv = small.tile([BLOC, WIN], FP)
                    nc.vector.tensor_tensor(out=smv[:], in0=smc[:],
                                            in1=valid_sl, op=ALU.mult)
                    mx = small.tile([BLOC, 1], FP)
                    nc.vector.tensor_reduce(out=mx[:], in_=smv[:],
                                            axis=AX.X, op=ALU.max)

            # pos col: ln(mx) per batch
            nc.scalar.activation(out=C[0:BLOC, NCH + 1:NCH + 2], in_=mx[:],
                                 func=AF.Ln)

            # ---- final: tot = sum over columns of wrow * colsum ----
            tot_ps = psum.tile([1, NCOL], FP)
            nc.tensor.matmul(out=tot_ps[:], lhsT=ones[:], rhs=C[:],
                             start=True, stop=True)
            negrow = small.tile([1, NCOL], FP)
            nc.vector.tensor_tensor(out=negrow[:], in0=tot_ps[:],
                                    in1=wrow_sl, op=ALU.mult)
            tot = small.tile([1, 1], FP)
            nc.vector.tensor_reduce(out=tot[:], in_=negrow[:], axis=AX.X,
                                    op=ALU.add)
            nc.gpsimd.dma_start(out=outd, in_=tot[:])

    nc.compile()
    _NC_CACHE = nc
    return nc


def _make_in_maps(X, lengths, tgt, w_end):
    X = np.asarray(X, dtype=np.float32)
    lengths = np.asarray(lengths, dtype=np.int64)
    tgt = np.asarray(tgt, dtype=np.int64)
    w_end = np.asarray(w_end, dtype=np.int64)

    tau_s = np.maximum(0, w_end + OFFSET_D - WIN)
    tau_e = np.minimum(tau_s + WIN, lengths)
    Lw = tau_e - tau_s

    Mmat = _conv_matrix()

    # final-combine weights: big cols and pos get -1, excl gets +1
    # (C holds +sum ln everywhere; loss = -A + Ex - L)
    wrow = np.full(NCOL, -1.0, np.float32)
    wrow[NCH] = 1.0

    in_maps = []
    for cr in range(NCORES):
        bs = slice(cr * BLOC, (cr + 1) * BLOC)
        ls, ts, lw, tg = lengths[bs], tau_s[bs], Lw[bs], tgt[bs]

        # per-core X copy with the invalid tail zeroed
        Xc = np.array(X[bs])                     # (8, T, K) contiguous copy
        for b in range(BLOC):
            lb = int(ls[b])
            if lb < T:
                Xc[b, lb:] = 0.0

        valid8 = (np.arange(WIN)[None, :] < lw[:, None]).astype(np.float32)
        aux = np.zeros((WIN, AUXW), np.float32)
        aux[0:WIN, 0:WIN] = Mmat
        aux[0:BLOC, WIN:2 * WIN] = valid8
        aux[0:BLOC, 2 * WIN:2 * WIN + BLOC] = np.eye(BLOC, dtype=np.float32)
        aux[0, 2 * WIN + BLOC:AUXW] = wrow

        oh = np.zeros((BLOC, K), np.float32)
        oh[np.arange(BLOC), tg] = 1.0
        ohrep = np.broadcast_to(oh[:, None, :], (BLOC, WIN, K)) \
            .reshape(BLOC, GLEN)

        gofs_arr = (np.arange(BLOC) * T + ts).astype(np.int32) \
            .reshape(BLOC, 1)
        in_maps.append({
            "Xs": Xc.reshape(P, FTOT),
            "aux": aux,
            "auxO": np.ascontiguousarray(ohrep),
            "gofs": gofs_arr,
        })
    return in_maps


def kernel(X, lengths, tgt, w_end):
    nc = _build_program()
    in_maps = _make_in_maps(X, lengths, tgt, w_end)
    res = bass_utils.run_bass_kernel_spmd(
        nc, in_maps, core_ids=list(range(NCORES)))
    total = np.float32(0.0)
    for c in range(NCORES):
        total += np.float32(res.results[c]["out"][0, 0])
    return np.array(total, dtype=np.float32)
